# revision 21
# baseline (speedup 1.0000x reference)
"""MoE gate routing kernel for Trainium2 (8 NeuronCores).

Computes the DeepSeek-style MoE gate of reference.py:
  gates = x @ W.T ; scores = sigmoid(gates) ; s = scores + bias
  group top-2 sums -> keep top-4 of 8 groups -> top-8 experts of masked s
  sel = normalized unbiased scores * 2.5
Returns (inds int32 [4,4096,8], sel float32 [4,4096,8]).

Sharding: data-parallel over tokens; each of the 8 cores routes 2048 tokens.
Host prep: x is transposed per-core to [H, tokens] and split into fp16
hi/lo pairs (lo scaled by 2^11) so the PE computes fp32-accurate gates with
3 fp16-rate matmul terms: x@W ~= xh@wh + 2^-11*(xh@wl' + xl'@wh),
wl' = (w-wh)*2^11, xl' = (x-xh)*2^11.  hi-term and lo-terms accumulate in
separate PSUM groups; a DVE op recombines them.
"""
import numpy as np

B, S, H, E = 4, 4096, 4096, 256
NCORES = 8
T = B * S
TPC = T // NCORES          # tokens per core
PT = 128                   # tokens per tile (partition dim)
TILES = TPC // PT          # 16
KCH = H // 128             # 32 contraction chunks
G = 8                      # expert groups
EG = E // G                # experts per group
TOPK_GROUP = 4
TOP_K = 8
LO_SCALE = 2048.0          # 2^11
ROUTED_SCALING_FACTOR = 2.5

_CACHE = {}


def _build():
    import concourse.tile as tile
    from concourse import bacc, mybir

    F32 = mybir.dt.float32
    F16 = mybir.dt.float16
    I32 = mybir.dt.int32
    U32 = mybir.dt.uint32
    Alu = mybir.AluOpType

    nc = bacc.Bacc(None, target_bir_lowering=False)
    # x hi/lo are pre-permuted on host to per-tile SBUF layout:
    # [TILES*128, KCH*PT] where row = tile*128 + partition, col = k*PT + t
    # wcat is pre-permuted to [128, KCH*2E]: row = h%128, col = (h//128)*2E + e
    xh_d = nc.dram_tensor("xh", [TPC, KCH * PT], F16, kind="ExternalInput")
    xl_d = nc.dram_tensor("xl", [TPC, KCH * PT], F16, kind="ExternalInput")
    w_d = nc.dram_tensor("wcat", [128, KCH * 2 * E], F16, kind="ExternalInput")
    b_d = nc.dram_tensor("bias", [1, E], F32, kind="ExternalInput")
    # outputs in [partition, tile, k] layout (contiguous per partition);
    # host reassembles to token order
    inds_d = nc.dram_tensor("inds", [128, TILES * TOP_K], I32,
                            kind="ExternalOutput")
    sel_d = nc.dram_tensor("sel", [128, TILES * TOP_K], F32,
                           kind="ExternalOutput")

    xh_v = xh_d.rearrange("(n p) f -> n p f", p=128)
    xl_v = xl_d.rearrange("(n p) f -> n p f", p=128)

    with tile.TileContext(nc) as tc:
        with (
            tc.tile_pool(name="const", bufs=1) as cpool,
            tc.tile_pool(name="xload", bufs=3) as xpool,
            tc.tile_pool(name="work", bufs=2) as pool,
            tc.tile_pool(name="out", bufs=1) as opool,
            tc.tile_pool(name="psum", bufs=2, space="PSUM") as pp,
        ):
            w_sb = cpool.tile([128, KCH, 2 * E], F16, tag="w")
            WC = 4  # k-chunks per W DMA

            def load_w_chunk(c):
                nc.sync.dma_start(
                    w_sb[:, c * WC:(c + 1) * WC, :].rearrange("p k e -> p (k e)"),
                    w_d[:, c * WC * 2 * E:(c + 1) * WC * 2 * E])

            # critical path first: W chunk 0, then tile-0 x loads (emitted by
            # the tt=0 iteration below), then the rest of W
            load_w_chunk(0)
            bias_bc = cpool.tile([128, E], F32, tag="bias")
            nc.sync.dma_start(bias_bc, b_d[:].to_broadcast([128, E]))

            inds_st = opool.tile([128, TILES, TOP_K], I32, tag="inds_st")
            sel_st = opool.tile([128, TILES, TOP_K], F32, tag="sel_st")

            for tt in range(TILES):
                HK = KCH // 2 * PT
                xh_t = xpool.tile([128, KCH, PT], F16, tag="xh")
                nc.sync.dma_start(
                    xh_t[:, :KCH // 2, :].rearrange("p k t -> p (k t)"),
                    xh_v[tt][:, :HK])
                nc.sync.dma_start(
                    xh_t[:, KCH // 2:, :].rearrange("p k t -> p (k t)"),
                    xh_v[tt][:, HK:])
                xl_t = xpool.tile([128, KCH, PT], F16, tag="xl")
                nc.sync.dma_start(
                    xl_t[:, :KCH // 2, :].rearrange("p k t -> p (k t)"),
                    xl_v[tt][:, :HK])
                nc.sync.dma_start(
                    xl_t[:, KCH // 2:, :].rearrange("p k t -> p (k t)"),
                    xl_v[tt][:, HK:])
                if tt == 0:
                    for c in range(1, KCH // WC):
                        load_w_chunk(c)

                # pA[:, :E] accumulates xh@wh ; pA[:, E:] accumulates
                # xh@wl' + xl@wh (both lo-terms share the 2^11 scale)
                pA = pp.tile([128, 2 * E], F32, tag="pA")
                for k in range(KCH):
                    nc.tensor.matmul(pA, xh_t[:, k, :], w_sb[:, k, :],
                                     start=(k == 0), stop=False)
                for k in range(KCH):
                    nc.tensor.matmul(pA[:, E:], xl_t[:, k, :], w_sb[:, k, :E],
                                     start=False, stop=(k == KCH - 1))

                # gates = pA[:, :E] + pA[:, E:] / 2^11
                # (DVE may read only one PSUM operand per op)
                tmp = pool.tile([128, E], F32, tag="tmp")
                nc.vector.tensor_scalar(
                    tmp, pA[:, E:], 1.0 / LO_SCALE, None, op0=Alu.mult)
                gates = pool.tile([128, E], F32, tag="gates")
                nc.vector.tensor_add(gates, pA[:, :E], tmp)

                # scores = sigmoid(gates); s = scores + bias
                scores = pool.tile([128, E], F32, tag="scores")
                nc.scalar.activation(scores, gates,
                                     mybir.ActivationFunctionType.Sigmoid)
                s = pool.tile([128, E], F32, tag="s")
                nc.vector.tensor_add(s, scores, bias_bc)

                # group scores: sum of top-2 within each group of 32
                gm = pool.tile([128, G, 8], F32, tag="gm")
                for g in range(G):
                    nc.vector.max(out=gm[:, g, :], in_=s[:, g * EG:(g + 1) * EG])
                gsc = pool.tile([128, G], F32, tag="gsc")
                nc.vector.tensor_add(gsc, gm[:, :, 0], gm[:, :, 1])

                # keep top-4 groups; sm = s where group kept else 0
                gsort = pool.tile([128, 8], F32, tag="gsort")
                nc.vector.max(out=gsort, in_=gsc)
                sm = pool.tile([128, E], F32, tag="sm")
                nc.vector.scalar_tensor_tensor(
                    sm.rearrange("p (g j) -> p g j", g=G),
                    gsc.unsqueeze(2).broadcast_to([128, G, EG]),
                    gsort[:, TOPK_GROUP - 1:TOPK_GROUP],
                    s.rearrange("p (g j) -> p g j", g=G),
                    op0=Alu.is_ge, op1=Alu.mult)

                # top-8 experts by biased score
                vals8 = pool.tile([128, 8], F32, tag="vals8")
                nc.vector.max(out=vals8, in_=sm)
                idxp = pool.tile([128, 16], U32, tag="idxp")
                nc.vector.max_index(idxp[:, :8], vals8, sm)

                # unbiased scores of the selected 8 (unbiased descending order)
                ssel = pool.tile([128, E], F32, tag="ssel")
                nc.vector.scalar_tensor_tensor(
                    ssel, sm, vals8[:, 7:8], scores,
                    op0=Alu.is_ge, op1=Alu.mult)
                u8 = pool.tile([128, 8], F32, tag="u8")
                nc.vector.max(out=u8, in_=ssel)
                nc.vector.max_index(idxp[:, 8:], u8, ssel)

                # realign unbiased values to biased rank order (8x8 match);
                # fused: prod3 = eq * u8, den = 1e-20 + sum(prod3)
                idxf = pool.tile([128, 16], F32, tag="idxf")
                nc.vector.tensor_copy(idxf, idxp)
                eq3 = pool.tile([128, 8, 8], F32, tag="eq3")
                nc.vector.tensor_tensor(
                    eq3,
                    idxf[:, :8].unsqueeze(2).broadcast_to([128, 8, 8]),
                    idxf[:, 8:].unsqueeze(1).broadcast_to([128, 8, 8]),
                    op=Alu.is_equal)
                prod3 = pool.tile([128, 8, 8], F32, tag="prod3")
                nc.vector.tensor_tensor(
                    prod3, eq3, u8.unsqueeze(1).broadcast_to([128, 8, 8]),
                    op=Alu.mult)
                selr = pool.tile([128, 8], F32, tag="selr")
                nc.vector.reduce_sum(selr, prod3, axis=mybir.AxisListType.X)
                den = pool.tile([128, 1], F32, tag="den")
                nc.vector.reduce_sum(den, selr, axis=mybir.AxisListType.X)
                nc.vector.tensor_scalar_add(den, den, 1e-20)

                # sel = selr * 2.5 / (sum + 1e-20)
                denr = pool.tile([128, 1], F32, tag="denr")
                nc.vector.reciprocal(denr, den)
                nc.vector.scalar_tensor_tensor(
                    sel_st[:, tt, :], selr, ROUTED_SCALING_FACTOR,
                    denr.to_broadcast([128, 8]), op0=Alu.mult, op1=Alu.mult)
                nc.vector.tensor_copy(inds_st[:, tt, :], idxp[:, :8])

                if tt == TILES // 2 - 1:
                    HF = TILES // 2 * TOP_K
                    nc.sync.dma_start(
                        inds_d[:, :HF],
                        inds_st[:, :TILES // 2, :].rearrange("p n k -> p (n k)"))
                    nc.sync.dma_start(
                        sel_d[:, :HF],
                        sel_st[:, :TILES // 2, :].rearrange("p n k -> p (n k)"))

            HF = TILES // 2 * TOP_K
            nc.sync.dma_start(
                inds_d[:, HF:],
                inds_st[:, TILES // 2:, :].rearrange("p n k -> p (n k)"))
            nc.sync.dma_start(
                sel_d[:, HF:],
                sel_st[:, TILES // 2:, :].rearrange("p n k -> p (n k)"))

    nc.compile()
    return nc


def _prep_inputs(x, weight, bias):
    """Host-side shard + transpose + fp16 hi/lo split."""
    xf = np.ascontiguousarray(x.reshape(T, H))
    wT = np.ascontiguousarray(weight.T.astype(np.float32))   # [H, E]
    wh = wT.astype(np.float16)
    wl = ((wT - wh.astype(np.float32)) * LO_SCALE).astype(np.float16)
    wcat = np.concatenate([wh, wl], axis=1)                  # [H, 2E]
    # permute to [128, KCH*2E]: row = h%128, col-major by k-chunk
    wcat = np.ascontiguousarray(
        wcat.reshape(KCH, 128, 2 * E).transpose(1, 0, 2).reshape(128, -1))
    b2 = np.ascontiguousarray(bias.astype(np.float32)[None, :])

    in_maps = []
    for c in range(NCORES):
        xc = xf[c * TPC:(c + 1) * TPC]                     # [TPC, H] f32
        # device tile layout: row = tile*128 + partition(=h%128? no: h-chunk
        # partition), col = k*PT + t ; element (tile,p,k,t) = x[tile*PT+t,
        # k*128+p] — i.e. x^T arranged per-tile contiguous.
        xt = xc.T.reshape(KCH, 128, TILES, PT)             # [k, p, tile, t]
        xt = np.ascontiguousarray(xt.transpose(2, 1, 0, 3))  # [tile, p, k, t]
        xt = xt.reshape(TPC, KCH * PT)
        xh = xt.astype(np.float16)
        xl = ((xt - xh.astype(np.float32)) * LO_SCALE).astype(np.float16)
        in_maps.append({"xh": xh, "xl": xl, "wcat": wcat, "bias": b2})
    return in_maps


def kernel(x, weight, bias):
    from concourse.bass_utils import run_bass_kernel_spmd

    if "nc" not in _CACHE:
        _CACHE["nc"] = _build()
    nc = _CACHE["nc"]

    in_maps = _prep_inputs(np.asarray(x), np.asarray(weight), np.asarray(bias))
    res = run_bass_kernel_spmd(nc, in_maps, core_ids=list(range(NCORES)))

    def unpack(a):
        # [128, TILES*TOP_K] -> [TILES*128, TOP_K] token order
        return a.reshape(128, TILES, TOP_K).transpose(1, 0, 2).reshape(
            TPC, TOP_K)

    inds = np.concatenate([unpack(r["inds"]) for r in res.results], axis=0)
    sel = np.concatenate([unpack(r["sel"]) for r in res.results], axis=0)
    return (inds.reshape(B, S, TOP_K).astype(np.int32),
            sel.reshape(B, S, TOP_K).astype(np.float32))


# revision 24
# speedup vs baseline: 1.0197x; 1.0197x over previous
"""MoE gate routing kernel for Trainium2 (8 NeuronCores).

Computes the DeepSeek-style MoE gate of reference.py:
  gates = x @ W.T ; scores = sigmoid(gates) ; s = scores + bias
  group top-2 sums -> keep top-4 of 8 groups -> top-8 experts of masked s
  sel = normalized unbiased scores * 2.5
Returns (inds int32 [4,4096,8], sel float32 [4,4096,8]).

Sharding: data-parallel over tokens; each of the 8 cores routes 2048 tokens.
Host prep: x is transposed per-core to [H, tokens] and split into fp16
hi/lo pairs (lo scaled by 2^11) so the PE computes fp32-accurate gates with
3 fp16-rate matmul terms: x@W ~= xh@wh + 2^-11*(xh@wl' + xl'@wh),
wl' = (w-wh)*2^11, xl' = (x-xh)*2^11.  hi-term and lo-terms accumulate in
separate PSUM groups; a DVE op recombines them.
"""
import numpy as np

B, S, H, E = 4, 4096, 4096, 256
NCORES = 8
T = B * S
TPC = T // NCORES          # tokens per core
PT = 128                   # tokens per tile (partition dim)
TILES = TPC // PT          # 16
KCH = H // 128             # 32 contraction chunks
G = 8                      # expert groups
EG = E // G                # experts per group
TOPK_GROUP = 4
TOP_K = 8
LO_SCALE = 2048.0          # 2^11
ROUTED_SCALING_FACTOR = 2.5

_CACHE = {}


def _build():
    import concourse.tile as tile
    from concourse import bacc, mybir

    F32 = mybir.dt.float32
    F16 = mybir.dt.float16
    I32 = mybir.dt.int32
    U32 = mybir.dt.uint32
    Alu = mybir.AluOpType

    nc = bacc.Bacc(None, target_bir_lowering=False)
    # x hi/lo are pre-permuted on host to per-tile SBUF layout:
    # [TILES*128, KCH*PT] where row = tile*128 + partition, col = k*PT + t
    # wcat is pre-permuted to [128, KCH*2E]: row = h%128, col = (h//128)*2E + e
    xh_d = nc.dram_tensor("xh", [TPC, KCH * PT], F16, kind="ExternalInput")
    xl_d = nc.dram_tensor("xl", [TPC, KCH * PT], F16, kind="ExternalInput")
    w_d = nc.dram_tensor("wcat", [128, KCH * 2 * E], F16, kind="ExternalInput")
    b_d = nc.dram_tensor("bias", [1, E], F32, kind="ExternalInput")
    # outputs in [partition, tile, k] layout (contiguous per partition);
    # host reassembles to token order
    inds_d = nc.dram_tensor("inds", [128, TILES * TOP_K], I32,
                            kind="ExternalOutput")
    sel_d = nc.dram_tensor("sel", [128, TILES * TOP_K], F32,
                           kind="ExternalOutput")

    xh_v = xh_d.rearrange("(n p) f -> n p f", p=128)
    xl_v = xl_d.rearrange("(n p) f -> n p f", p=128)

    with tile.TileContext(nc) as tc:
        with (
            tc.tile_pool(name="const", bufs=1) as cpool,
            tc.tile_pool(name="xload", bufs=4) as xpool,
            tc.tile_pool(name="work", bufs=3) as pool,
            tc.tile_pool(name="out", bufs=1) as opool,
            tc.tile_pool(name="psum", bufs=3, space="PSUM") as pp,
        ):
            w_sb = cpool.tile([128, KCH, 2 * E], F16, tag="w")
            WC = 4  # k-chunks per W DMA

            def load_w_chunk(c):
                nc.sync.dma_start(
                    w_sb[:, c * WC:(c + 1) * WC, :].rearrange("p k e -> p (k e)"),
                    w_d[:, c * WC * 2 * E:(c + 1) * WC * 2 * E])

            # critical path first: W chunk 0, then tile-0 x loads (emitted by
            # the tt=0 iteration below), then the rest of W
            load_w_chunk(0)
            bias_bc = cpool.tile([128, E], F32, tag="bias")
            nc.sync.dma_start(bias_bc, b_d[:].to_broadcast([128, E]))

            inds_st = opool.tile([128, TILES, TOP_K], I32, tag="inds_st")
            sel_st = opool.tile([128, TILES, TOP_K], F32, tag="sel_st")

            for tt in range(TILES):
                HK = KCH // 2 * PT
                xh_t = xpool.tile([128, KCH, PT], F16, tag="xh")
                nc.sync.dma_start(
                    xh_t[:, :KCH // 2, :].rearrange("p k t -> p (k t)"),
                    xh_v[tt][:, :HK])
                nc.sync.dma_start(
                    xh_t[:, KCH // 2:, :].rearrange("p k t -> p (k t)"),
                    xh_v[tt][:, HK:])
                # tile 0: the hi-term matmuls need all W chunks before xl is
                # touched, so emit W before the xl loads
                if tt == 0:
                    for c in range(1, KCH // WC):
                        load_w_chunk(c)
                xl_t = xpool.tile([128, KCH, PT], F16, tag="xl")
                nc.sync.dma_start(
                    xl_t[:, :KCH // 2, :].rearrange("p k t -> p (k t)"),
                    xl_v[tt][:, :HK])
                nc.sync.dma_start(
                    xl_t[:, KCH // 2:, :].rearrange("p k t -> p (k t)"),
                    xl_v[tt][:, HK:])

                # pA[:, :E] accumulates xh@wh ; pA[:, E:] accumulates
                # xh@wl' + xl@wh (both lo-terms share the 2^11 scale)
                pA = pp.tile([128, 2 * E], F32, tag="pA")
                for k in range(KCH):
                    nc.tensor.matmul(pA, xh_t[:, k, :], w_sb[:, k, :],
                                     start=(k == 0), stop=False)
                for k in range(KCH):
                    nc.tensor.matmul(pA[:, E:], xl_t[:, k, :], w_sb[:, k, :E],
                                     start=False, stop=(k == KCH - 1))

                # gates = pA[:, :E] + pA[:, E:] / 2^11
                # (DVE may read only one PSUM operand per op)
                tmp = pool.tile([128, E], F32, tag="tmp")
                nc.vector.tensor_scalar(
                    tmp, pA[:, E:], 1.0 / LO_SCALE, None, op0=Alu.mult)
                gates = pool.tile([128, E], F32, tag="gates")
                nc.vector.tensor_add(gates, pA[:, :E], tmp)

                # scores = sigmoid(gates); s = scores + bias
                scores = pool.tile([128, E], F32, tag="scores")
                nc.scalar.activation(scores, gates,
                                     mybir.ActivationFunctionType.Sigmoid)
                s = pool.tile([128, E], F32, tag="s")
                nc.vector.tensor_add(s, scores, bias_bc)

                # group scores: sum of top-2 within each group of 32
                gm = pool.tile([128, G, 8], F32, tag="gm")
                for g in range(G):
                    nc.vector.max(out=gm[:, g, :], in_=s[:, g * EG:(g + 1) * EG])
                gsc = pool.tile([128, G], F32, tag="gsc")
                nc.vector.tensor_add(gsc, gm[:, :, 0], gm[:, :, 1])

                # keep top-4 groups; sm = s where group kept else 0
                gsort = pool.tile([128, 8], F32, tag="gsort")
                nc.vector.max(out=gsort, in_=gsc)
                sm = pool.tile([128, E], F32, tag="sm")
                nc.vector.scalar_tensor_tensor(
                    sm.rearrange("p (g j) -> p g j", g=G),
                    gsc.unsqueeze(2).broadcast_to([128, G, EG]),
                    gsort[:, TOPK_GROUP - 1:TOPK_GROUP],
                    s.rearrange("p (g j) -> p g j", g=G),
                    op0=Alu.is_ge, op1=Alu.mult)

                # top-8 experts by biased score
                vals8 = pool.tile([128, 8], F32, tag="vals8")
                nc.vector.max(out=vals8, in_=sm)
                idxp = pool.tile([128, 16], U32, tag="idxp")
                nc.vector.max_index(idxp[:, :8], vals8, sm)

                # unbiased scores of the selected 8 (unbiased descending order)
                ssel = pool.tile([128, E], F32, tag="ssel")
                nc.vector.scalar_tensor_tensor(
                    ssel, sm, vals8[:, 7:8], scores,
                    op0=Alu.is_ge, op1=Alu.mult)
                u8 = pool.tile([128, 8], F32, tag="u8")
                nc.vector.max(out=u8, in_=ssel)
                nc.vector.max_index(idxp[:, 8:], u8, ssel)

                # realign unbiased values to biased rank order (8x8 match);
                # fused: prod3 = eq * u8, den = 1e-20 + sum(prod3)
                idxf = pool.tile([128, 16], F32, tag="idxf")
                nc.vector.tensor_copy(idxf, idxp)
                eq3 = pool.tile([128, 8, 8], F32, tag="eq3")
                nc.vector.tensor_tensor(
                    eq3,
                    idxf[:, :8].unsqueeze(2).broadcast_to([128, 8, 8]),
                    idxf[:, 8:].unsqueeze(1).broadcast_to([128, 8, 8]),
                    op=Alu.is_equal)
                prod3 = pool.tile([128, 8, 8], F32, tag="prod3")
                nc.vector.tensor_tensor(
                    prod3, eq3, u8.unsqueeze(1).broadcast_to([128, 8, 8]),
                    op=Alu.mult)
                selr = pool.tile([128, 8], F32, tag="selr")
                nc.vector.reduce_sum(selr, prod3, axis=mybir.AxisListType.X)
                den = pool.tile([128, 1], F32, tag="den")
                nc.vector.reduce_sum(den, selr, axis=mybir.AxisListType.X)
                nc.vector.tensor_scalar_add(den, den, 1e-20)

                # sel = selr * 2.5 / (sum + 1e-20)
                denr = pool.tile([128, 1], F32, tag="denr")
                nc.vector.reciprocal(denr, den)
                nc.vector.scalar_tensor_tensor(
                    sel_st[:, tt, :], selr, ROUTED_SCALING_FACTOR,
                    denr.to_broadcast([128, 8]), op0=Alu.mult, op1=Alu.mult)
                nc.vector.tensor_copy(inds_st[:, tt, :], idxp[:, :8])

                if tt == TILES // 2 - 1:
                    HF = TILES // 2 * TOP_K
                    nc.sync.dma_start(
                        inds_d[:, :HF],
                        inds_st[:, :TILES // 2, :].rearrange("p n k -> p (n k)"))
                    nc.sync.dma_start(
                        sel_d[:, :HF],
                        sel_st[:, :TILES // 2, :].rearrange("p n k -> p (n k)"))

            HF = TILES // 2 * TOP_K
            nc.sync.dma_start(
                inds_d[:, HF:],
                inds_st[:, TILES // 2:, :].rearrange("p n k -> p (n k)"))
            nc.sync.dma_start(
                sel_d[:, HF:],
                sel_st[:, TILES // 2:, :].rearrange("p n k -> p (n k)"))

    nc.compile()
    return nc


def _prep_inputs(x, weight, bias):
    """Host-side shard + transpose + fp16 hi/lo split."""
    xf = np.ascontiguousarray(x.reshape(T, H))
    wT = np.ascontiguousarray(weight.T.astype(np.float32))   # [H, E]
    wh = wT.astype(np.float16)
    wl = ((wT - wh.astype(np.float32)) * LO_SCALE).astype(np.float16)
    wcat = np.concatenate([wh, wl], axis=1)                  # [H, 2E]
    # permute to [128, KCH*2E]: row = h%128, col-major by k-chunk
    wcat = np.ascontiguousarray(
        wcat.reshape(KCH, 128, 2 * E).transpose(1, 0, 2).reshape(128, -1))
    b2 = np.ascontiguousarray(bias.astype(np.float32)[None, :])

    in_maps = []
    for c in range(NCORES):
        xc = xf[c * TPC:(c + 1) * TPC]                     # [TPC, H] f32
        # device layout [tile*128+p, k*PT+t] = x[tile*PT+t, k*128+p]:
        # x^T arranged so each per-tile DMA is contiguous per partition
        xt = xc.T.reshape(KCH, 128, TILES, PT)             # [k, p, tile, t]
        xt = np.ascontiguousarray(xt.transpose(2, 1, 0, 3))  # [tile, p, k, t]
        xt = xt.reshape(TPC, KCH * PT)
        xh = xt.astype(np.float16)
        xl = ((xt - xh.astype(np.float32)) * LO_SCALE).astype(np.float16)
        in_maps.append({"xh": xh, "xl": xl, "wcat": wcat, "bias": b2})
    return in_maps


def kernel(x, weight, bias):
    from concourse.bass_utils import run_bass_kernel_spmd

    if "nc" not in _CACHE:
        _CACHE["nc"] = _build()
    nc = _CACHE["nc"]

    in_maps = _prep_inputs(np.asarray(x), np.asarray(weight), np.asarray(bias))
    res = run_bass_kernel_spmd(nc, in_maps, core_ids=list(range(NCORES)))

    def unpack(a):
        # [128, TILES*TOP_K] -> [TILES*128, TOP_K] token order
        return a.reshape(128, TILES, TOP_K).transpose(1, 0, 2).reshape(
            TPC, TOP_K)

    inds = np.concatenate([unpack(r["inds"]) for r in res.results], axis=0)
    sel = np.concatenate([unpack(r["sel"]) for r in res.results], axis=0)
    return (inds.reshape(B, S, TOP_K).astype(np.int32),
            sel.reshape(B, S, TOP_K).astype(np.float32))
